# revision 3
# baseline (speedup 1.0000x reference)
"""LightGCN (AIM variant) distributed Bass kernel for 8 TRN2 NeuronCores.

Strategy (destination sharding):
  - 150000 nodes split into 8 slices of 18750 (padded to 18816 = 147*128 rows
    per slice; pad rows are always zero). Core k owns the destinations of
    slice k and all edges pointing into them (~500k edges/core).
  - z-substitution: with z = dis * x (dis = deg^-1/2), each LightGCN layer is
    x_{l+1}[c] = dis[c] * sum_{e in(c)} z_l[row_e]  -- a pure gather +
    segment-sum; the per-edge norm multiplier disappears.
  - Per layer: each core gathers z rows for its edges (dma_gather custom
    instruction, 4 SWDGE queues, int16 indices -> the 150528-row replicated
    table is split into 5 chunks of <=30112 rows), segment-sums them on the
    TensorEngine via on-device-built one-hot matrices (PSUM accumulation per
    128-destination group), scales by dis, and AllGathers the new z slices.
  - Epilogue: item rows L2-normalized (*1.5), final out table AllGathered,
    the 2x8192 label endpoints gathered, ranks + beta terms computed on
    device, and the (4096, 4096) outer-sum outputs written (rows sharded
    across cores).
"""

import numpy as np

import concourse.bass as bass
import concourse.bacc as bacc
import concourse.tile as tile
import concourse.mybir as mybir
from concourse.bass_utils import run_bass_kernel_spmd
from concourse.masks import make_identity

# ---------------------------------------------------------------- constants
N = 150000
D = 64
NLAYERS = 3
NLAB = 8192          # 2*B label pairs
B = 4096
NCORE = 8
SLICE_REAL = 18750
SLICE_PAD = 18816    # 147 * 128
G_GROUPS = 147
TABLE = NCORE * SLICE_PAD   # 150528
CHUNK = 30112
NCHUNK = 5
INSTR_MAX = 1024     # idxs per dma_gather instruction (hw-safe w/ single_packet)
IDXW = 4096          # idx window width in int16 columns (= 65536 idxs)
BETA_WIN = (N + 63) // 64   # 2344 windows of 64 floats
EPS = 1e-12

F32 = mybir.dt.float32
I16 = mybir.dt.int16


def _chunk_bounds():
    lo = [c * CHUNK for c in range(NCHUNK)]
    hi = [min((c + 1) * CHUNK, TABLE) for c in range(NCHUNK)]
    return lo, hi


def _zrow_locals():
    lo, hi = _chunk_bounds()
    out = []
    for c in range(NCHUNK):
        z = None
        for s in range(NCORE):
            zr = s * SLICE_PAD + SLICE_REAL
            if lo[c] <= zr < hi[c]:
                z = zr - lo[c]
                break
        assert z is not None
        out.append(z)
    return out


def _wrap16(flat):
    """int16 stream -> [128, len/16] wrapped (idx i at [i%16, i//16]) and
    replicated across the 8 16-partition groups."""
    w = flat.reshape(-1, 16).T
    return np.tile(w, (8, 1)).copy()


# ---------------------------------------------------------------- host prep
def _prep(emb_weight, beta_weight, alpha, edge_index, edge_label_index,
          num_users, scaling_factor):
    row = np.asarray(edge_index[0]).astype(np.int64)
    col = np.asarray(edge_index[1]).astype(np.int64)
    E = row.shape[0]
    num_users = int(num_users)

    deg = np.bincount(col, minlength=N).astype(np.int64)
    r_tab = (row // SLICE_REAL) * SLICE_PAD + (row % SLICE_REAL)
    core_of = col // SLICE_REAL
    zrl = _zrow_locals()

    # ---- per-core sorted edge structure + shared cell sizes
    per_core = []
    counts = np.zeros((NCORE, G_GROUPS, NCHUNK), np.int64)
    for k in range(NCORE):
        m = core_of == k
        ld = col[m] - k * SLICE_REAL
        rt = r_tab[m]
        ch = rt // CHUNK
        g = ld // 128
        order = np.lexsort((ch, g))
        ld, rt, ch, g = ld[order], rt[order], ch[order], g[order]
        cell = g * NCHUNK + ch
        counts[k] = np.bincount(cell, minlength=G_GROUPS * NCHUNK).reshape(
            G_GROUPS, NCHUNK)
        per_core.append((ld, rt, cell))

    P = ((counts.max(axis=0) + 127) // 128) * 128        # [G, NCHUNK]
    cell_start = np.zeros(G_GROUPS * NCHUNK + 1, np.int64)
    cell_start[1:] = np.cumsum(P.reshape(-1))
    S = int(cell_start[-1])
    SCOLS = S // 128

    # instruction list (same for all cores): (chunk, n, idx_col, slot_off)
    instrs = []
    col_cur = 0
    slot_off = 0
    for g in range(G_GROUPS):
        for c in range(NCHUNK):
            rem = int(P[g, c])
            while rem > 0:
                n = min(INSTR_MAX, rem)
                ncols = n // 16
                if col_cur % IDXW + ncols > IDXW:
                    col_cur = (col_cur // IDXW + 1) * IDXW
                instrs.append((c, n, col_cur, slot_off))
                col_cur += ncols
                slot_off += n
                rem -= n
    TOTCOLS = ((col_cur + IDXW - 1) // IDXW) * IDXW
    NWIN = TOTCOLS // IDXW

    # map slot position -> idx position
    slot2idx = np.empty(S, np.int64)
    for c, n, icol, soff in instrs:
        slot2idx[soff:soff + n] = np.arange(icol * 16, icol * 16 + n)

    # group chunk ranges for matmuls: group g covers dsel cols
    # [cell_start[g*NCHUNK]//128, cell_start[(g+1)*NCHUNK]//128)
    group_col0 = [int(cell_start[g * NCHUNK] // 128) for g in range(G_GROUPS + 1)]

    lab_src = np.asarray(edge_label_index[0]).astype(np.int64)
    lab_dst = np.asarray(edge_label_index[1]).astype(np.int64)
    lab_nodes = np.concatenate([lab_src, lab_dst])       # 16384 slots
    lab_tab = (lab_nodes // SLICE_REAL) * SLICE_PAD + (lab_nodes % SLICE_REAL)

    alpha = np.asarray(alpha, np.float32).reshape(-1)
    emb_weight = np.asarray(emb_weight, np.float32)
    beta_flat = np.zeros(BETA_WIN * 64, np.float32)
    beta_flat[:N] = np.asarray(beta_weight, np.float32).reshape(-1)

    in_maps = []
    for k in range(NCORE):
        ld, rt, cell = per_core[k]
        nk = ld.shape[0]
        # slot of each edge: cell_start[cell] + rank within cell
        first_idx = np.zeros(G_GROUPS * NCHUNK, np.int64)
        cnt = counts[k].reshape(-1)
        first_idx[1:] = np.cumsum(cnt)[:-1]
        pos_in_cell = np.arange(nk) - np.repeat(first_idx, cnt)
        slot = cell_start[cell] + pos_in_cell

        # idx stream (pads = chunk zero row)
        idx_flat = np.zeros(TOTCOLS * 16, np.int16)
        for c, n, icol, soff in instrs:
            idx_flat[icol * 16: icol * 16 + n] = zrl[c]
        idx_flat[slot2idx[slot]] = (rt - (cell % NCHUNK) * CHUNK).astype(np.int16)

        # dsel stream (pads = -1)
        dsel = np.full(S, -1.0, np.float32)
        dsel[slot] = (ld % 128).astype(np.float32)

        # per-dest metadata in [p, g] layout (dest local id = g*128 + p)
        degs = np.zeros(SLICE_PAD, np.float32)
        degs[:SLICE_REAL] = deg[k * SLICE_REAL:(k + 1) * SLICE_REAL]
        mdeg = (degs > 0).astype(np.float32)
        ids = np.arange(k * SLICE_REAL, k * SLICE_REAL + SLICE_PAD)
        mitem = ((ids >= num_users) & (ids < k * SLICE_REAL + SLICE_REAL)).astype(np.float32)

        embs = np.zeros((SLICE_PAD, D), np.float32)
        embs[:SLICE_REAL] = emb_weight[k * SLICE_REAL:(k + 1) * SLICE_REAL]

        # label gather streams
        lab_idx = np.empty((128, NCHUNK * (NLAB * 2 // 16) // 8), np.int16)
        lab_parts = []
        for c in range(NCHUNK):
            v = np.where(lab_tab // CHUNK == c, lab_tab - c * CHUNK,
                         zrl[c]).astype(np.int16)
            lab_parts.append(_wrap16(v))
        lab_idx = np.concatenate(lab_parts, axis=1)      # [128, 5*1024]

        bwin = _wrap16((lab_dst // 64).astype(np.int16))  # [128, 512]
        e8 = np.zeros((NLAB, 64), np.float32)
        e8[np.arange(NLAB), lab_dst % 64] = 1.0
        e8 = e8.reshape(64, 128, 64).transpose(1, 0, 2).copy()  # [128,64,64]

        # outer-sum beta column selectors: output row block t (0..3) of this
        # core selects beta_d column k*4+t (pos) / 32+k*4+t (neg)
        osel = np.zeros((8, 64), np.float32)
        for t in range(4):
            osel[t, k * 4 + t] = 1.0
            osel[4 + t, 32 + k * 4 + t] = 1.0
        osel = np.tile(osel[None, :, :], (128, 1, 1)).copy()

        in_maps.append({
            "emb": embs,
            "degf": degs.reshape(G_GROUPS, 128).T.copy(),
            "mdeg": mdeg.reshape(G_GROUPS, 128).T.copy(),
            "mitem": mitem.reshape(G_GROUPS, 128).T.copy(),
            "alpha": np.tile(alpha.reshape(1, 4), (128, 1)),
            "scal": np.full((128, 1), float(scaling_factor), np.float32),
            "idx": _wrap16(idx_flat),
            "dsel": dsel.reshape(-1, 128).T.copy(),
            "lab": lab_idx,
            "bwin": bwin,
            "e8": e8.reshape(128, 64 * 64),
            "beta": beta_flat.reshape(BETA_WIN, 64),
            "iota": np.tile(np.arange(128, dtype=np.float32)[None, :], (128, 1)),
            "osel": osel.reshape(128, 8 * 64),
        })

    meta = dict(instrs=instrs, S=S, SCOLS=SCOLS, TOTCOLS=TOTCOLS, NWIN=NWIN,
                group_col0=group_col0)
    return in_maps, meta


# ---------------------------------------------------------------- builder
def _bc_ap(base_ap, p_count, mid_count, last_count, mid_step, last_step):
    """Construct an AP [p_count, mid_count, last_count] over base_ap's tensor."""
    return bass.AP(base_ap.tensor, base_ap.offset,
                   [list(base_ap.ap[0])[:1] + [p_count],
                    [mid_step, mid_count],
                    [last_step, last_count]])


def _build(meta):
    instrs = meta["instrs"]
    SCOLS = meta["SCOLS"]
    TOTCOLS = meta["TOTCOLS"]
    group_col0 = meta["group_col0"]
    clo, chi = _chunk_bounds()

    nc = bacc.Bacc(None, target_bir_lowering=False, num_swdge_queues=4)
    dp = nc.declare_dram_parameter
    emb_e = dp("emb", [SLICE_PAD, D], F32, isOutput=False)
    degf_e = dp("degf", [128, G_GROUPS], F32, isOutput=False)
    mdeg_e = dp("mdeg", [128, G_GROUPS], F32, isOutput=False)
    mitem_e = dp("mitem", [128, G_GROUPS], F32, isOutput=False)
    alpha_e = dp("alpha", [128, 4], F32, isOutput=False)
    scal_e = dp("scal", [128, 1], F32, isOutput=False)
    idx_e = dp("idx", [128, TOTCOLS], I16, isOutput=False)
    dsel_e = dp("dsel", [128, SCOLS], F32, isOutput=False)
    lab_e = dp("lab", [128, NCHUNK * 1024], I16, isOutput=False)
    bwin_e = dp("bwin", [128, 512], I16, isOutput=False)
    e8_e = dp("e8", [128, 64 * 64], F32, isOutput=False)
    beta_e = dp("beta", [BETA_WIN, 64], F32, isOutput=False)
    iota_e = dp("iota", [128, 128], F32, isOutput=False)
    osel_e = dp("osel", [128, 8 * 64], F32, isOutput=False)
    out_e = dp("out", [2, 512, 4096], F32, isOutput=True)

    zslice = [nc.dram_tensor(f"zs{l}", [SLICE_PAD, D], F32) for l in range(NLAYERS)]
    zfull = [nc.dram_tensor(f"zf{l}", [TABLE, D], F32, addr_space="Shared")
             for l in range(NLAYERS)]
    oslice = nc.dram_tensor("oslice", [SLICE_PAD, D], F32)
    ofull = nc.dram_tensor("ofull", [TABLE, D], F32, addr_space="Shared")
    rrow_d = nc.dram_tensor("rrow", [1, NLAB], F32)

    rg = [list(range(NCORE))]
    mul = mybir.AluOpType.mult
    add = mybir.AluOpType.add

    with tile.TileContext(nc) as tc:
        with (
            tc.tile_pool(name="persist", bufs=1) as pp,
            tc.tile_pool(name="psum", bufs=4, space="PSUM") as psp,
        ):
            out_acc = pp.tile([128, G_GROUPS, D], F32)
            dis = pp.tile([128, G_GROUPS], F32)
            alpha_sb = pp.tile([128, 4], F32)
            scal_sb = pp.tile([128, 1], F32)
            mitem_sb = pp.tile([128, G_GROUPS], F32)

            nc.sync.dma_start(out=alpha_sb[:], in_=alpha_e[:, :])
            nc.sync.dma_start(out=scal_sb[:], in_=scal_e[:, :])
            nc.sync.dma_start(out=mitem_sb[:], in_=mitem_e[:, :])
            _layers(nc, tc, meta, locals())
            _epilogue(nc, tc, meta, locals())
    return nc


def _layers(nc, tc, meta, env):
    instrs = meta["instrs"]
    SCOLS = meta["SCOLS"]
    group_col0 = meta["group_col0"]
    clo, chi = _chunk_bounds()
    mul = mybir.AluOpType.mult
    out_acc = env["out_acc"]; dis = env["dis"]; alpha_sb = env["alpha_sb"]
    psp = env["psp"]
    emb_e = env["emb_e"]; degf_e = env["degf_e"]; mdeg_e = env["mdeg_e"]
    dsel_e = env["dsel_e"]; iota_e = env["iota_e"]; idx_e = env["idx_e"]
    zslice = env["zslice"]; zfull = env["zfull"]
    rg = [list(range(NCORE))]
    with (
        tc.tile_pool(name="work", bufs=3) as wp,
        tc.tile_pool(name="gat", bufs=12) as gp,
        tc.tile_pool(name="bmat", bufs=6) as bp,
        tc.tile_pool(name="idxw", bufs=2) as ip,
        tc.tile_pool(name="lpersist", bufs=1) as lp,
    ):
            dsel_sb = lp.tile([128, SCOLS], F32)
            iota_sb = lp.tile([128, 128], F32)
            nc.sync.dma_start(out=dsel_sb[:], in_=dsel_e[:, :])
            nc.sync.dma_start(out=iota_sb[:], in_=iota_e[:, :])

            # ---- dis = (deg > 0) / sqrt(max(deg, 1))
            degf = wp.tile([128, G_GROUPS], F32, tag="deg")
            mdeg = wp.tile([128, G_GROUPS], F32, tag="deg")
            nc.sync.dma_start(out=degf[:], in_=degf_e[:, :])
            nc.sync.dma_start(out=mdeg[:], in_=mdeg_e[:, :])
            degc = wp.tile([128, G_GROUPS], F32, tag="deg")
            nc.vector.tensor_scalar_max(out=degc[:], in0=degf[:], scalar1=1.0)
            dsq = wp.tile([128, G_GROUPS], F32, tag="deg")
            nc.scalar.activation(out=dsq[:], in_=degc[:],
                                 func=mybir.ActivationFunctionType.Sqrt)
            drc = wp.tile([128, G_GROUPS], F32, tag="deg")
            nc.vector.reciprocal(out=drc[:], in_=dsq[:])
            nc.vector.tensor_mul(out=dis[:], in0=drc[:], in1=mdeg[:])

            # ---- z0 slice + out_acc init
            for g in range(G_GROUPS):
                et = wp.tile([128, D], F32, tag="emb")
                nc.sync.dma_start(out=et[:], in_=emb_e[g * 128:(g + 1) * 128, :])
                z0 = wp.tile([128, D], F32, tag="z0")
                nc.vector.tensor_scalar(out=z0[:], in0=et[:],
                                        scalar1=dis[:, g:g + 1], scalar2=None,
                                        op0=mul)
                nc.sync.dma_start(out=zslice[0][g * 128:(g + 1) * 128, :], in_=z0[:])
                nc.vector.tensor_scalar(out=out_acc[:, g, :], in0=et[:],
                                        scalar1=alpha_sb[:, 0:1], scalar2=None,
                                        op0=mul)
            nc.gpsimd.collective_compute(
                "AllGather", mybir.AluOpType.bypass, replica_groups=rg,
                ins=[zslice[0].ap().opt()], outs=[zfull[0].ap().opt()])

            # ---- propagation layers
            qrr = 0
            for l in range(NLAYERS):
                zf = zfull[l]
                win_tile = None
                win_id = -1
                ii = 0  # instruction cursor
                for g in range(G_GROUPS):
                    psum = psp.tile([128, D], F32)
                    ncols_g = group_col0[g + 1] - group_col0[g]
                    done_cols = 0
                    while done_cols < ncols_g:
                        c, n, icol, soff = instrs[ii]
                        ii += 1
                        nb = n // 128
                        w = icol // IDXW
                        if w != win_id:
                            win_tile = ip.tile([128, IDXW], I16, tag="idxw")
                            nc.sync.dma_start(
                                out=win_tile[:],
                                in_=idx_e[:, w * IDXW:(w + 1) * IDXW])
                            win_id = w
                        gt = gp.tile([128, 8, D], F32, tag="gat")
                        wc = icol % IDXW
                        nc.gpsimd.dma_gather(
                            out_ap=gt[:, :nb, :],
                            in_ap=zf[clo[c]:chi[c], :],
                            idxs_ap=win_tile[:, wc:wc + n // 16],
                            num_idxs=n,
                            num_idxs_reg=n,
                            elem_size=D,
                            single_packet=True,
                            queue_num=qrr % 4,
                        )
                        qrr += 1
                        # one-hot matrices for the nb chunks
                        bt = bp.tile([128, 8, 128], F32, tag="bmat")
                        col0 = soff // 128
                        in0 = dsel_sb[:, col0:col0 + nb].to_broadcast([128, nb, 128])
                        in1 = _bc_ap(iota_sb[:], 128, nb, 128, 0, 1)
                        nc.vector.tensor_tensor(out=bt[:, :nb, :], in0=in0, in1=in1,
                                                op=mybir.AluOpType.is_equal)
                        for j in range(nb):
                            nc.tensor.matmul(
                                psum[:], bt[:, j, :], gt[:, j, :],
                                start=(done_cols + j == 0),
                                stop=(done_cols + j == ncols_g - 1))
                        done_cols += nb
                    # psum -> x_{l+1} handling
                    if l < NLAYERS - 1:
                        zn = wp.tile([128, D], F32, tag="zn")
                        nc.vector.tensor_scalar(out=zn[:], in0=psum[:],
                                                scalar1=dis[:, g:g + 1],
                                                scalar2=dis[:, g:g + 1],
                                                op0=mul, op1=mul)
                        nc.sync.dma_start(
                            out=zslice[l + 1][g * 128:(g + 1) * 128, :], in_=zn[:])
                    t2 = wp.tile([128, D], F32, tag="t2")
                    nc.vector.tensor_scalar(out=t2[:], in0=psum[:],
                                            scalar1=dis[:, g:g + 1],
                                            scalar2=alpha_sb[:, l + 1:l + 2],
                                            op0=mul, op1=mul)
                    nc.vector.tensor_add(out=out_acc[:, g, :],
                                         in0=out_acc[:, g, :], in1=t2[:])
                assert group_col0[G_GROUPS] * 128 == meta["S"]
                if l < NLAYERS - 1:
                    nc.gpsimd.collective_compute(
                        "AllGather", mybir.AluOpType.bypass, replica_groups=rg,
                        ins=[zslice[l + 1].ap().opt()],
                        outs=[zfull[l + 1].ap().opt()])


def _epilogue(nc, tc, meta, env):
    clo, chi = _chunk_bounds()
    mul = mybir.AluOpType.mult
    add = mybir.AluOpType.add
    out_acc = env["out_acc"]; dis = env["dis"]; scal_sb = env["scal_sb"]
    mitem_sb = env["mitem_sb"]; psp = env["psp"]
    oslice = env["oslice"]; ofull = env["ofull"]; rrow_d = env["rrow_d"]
    lab_e = env["lab_e"]; bwin_e = env["bwin_e"]; e8_e = env["e8_e"]
    beta_e = env["beta_e"]; osel_e = env["osel_e"]; out_e = env["out_e"]
    pp = env["pp"]
    rg = [list(range(NCORE))]
    with (
        tc.tile_pool(name="ep", bufs=1) as ep,
        tc.tile_pool(name="ew", bufs=3) as wp,
        tc.tile_pool(name="eg", bufs=2) as gp,
        tc.tile_pool(name="orow", bufs=2) as op,
    ):
            # ---- item normalization + out slice
            for g in range(G_GROUPS):
                sq = wp.tile([128, D], F32, tag="sq")
                nc.vector.tensor_mul(out=sq[:], in0=out_acc[:, g, :],
                                     in1=out_acc[:, g, :])
                ss = wp.tile([128, 1], F32, tag="ss")
                nc.vector.reduce_sum(ss[:], sq[:], axis=mybir.AxisListType.X)
                nrm = wp.tile([128, 1], F32, tag="ss")
                nc.scalar.activation(out=nrm[:], in_=ss[:],
                                     func=mybir.ActivationFunctionType.Sqrt)
                nc.vector.tensor_scalar_max(out=nrm[:], in0=nrm[:], scalar1=EPS)
                rec = wp.tile([128, 1], F32, tag="ss")
                nc.vector.reciprocal(out=rec[:], in_=nrm[:])
                fac = wp.tile([128, 1], F32, tag="ss")
                # fac = rec*scal - 1
                nc.vector.tensor_scalar(out=fac[:], in0=rec[:],
                                        scalar1=scal_sb[:, 0:1], scalar2=-1.0,
                                        op0=mul, op1=add)
                # fac = fac*mitem + 1
                nc.vector.tensor_scalar(out=fac[:], in0=fac[:],
                                        scalar1=mitem_sb[:, g:g + 1], scalar2=1.0,
                                        op0=mul, op1=add)
                on = wp.tile([128, D], F32, tag="on")
                nc.vector.tensor_scalar(out=on[:], in0=out_acc[:, g, :],
                                        scalar1=fac[:, 0:1], scalar2=None,
                                        op0=mul)
                nc.sync.dma_start(out=oslice[g * 128:(g + 1) * 128, :], in_=on[:])
            nc.gpsimd.collective_compute(
                "AllGather", mybir.AluOpType.bypass, replica_groups=rg,
                ins=[oslice.ap().opt()], outs=[ofull.ap().opt()])

            # ---- label gathers (16384 slots, 5 chunk passes summed)
            # acc_lab reuses the (now dead) out_acc slot in the persist pool
            lab_sb = ep.tile([128, NCHUNK * 1024], I16, tag="lab")
            nc.sync.dma_start(out=lab_sb[:], in_=lab_e[:, :])
            acc_lab = pp.tile([128, 128, D], F32, tag="out_acc")
            for c in range(NCHUNK):
                for half in range(2):
                    lt = gp.tile([128, 64, D], F32, tag="labg")
                    for piece in range(8):
                        pc = half * 8 + piece
                        nc.gpsimd.dma_gather(
                            out_ap=lt[:, piece * 8:(piece + 1) * 8, :],
                            in_ap=ofull[clo[c]:chi[c], :],
                            idxs_ap=lab_sb[:, c * 1024 + pc * 64:
                                           c * 1024 + (pc + 1) * 64],
                            num_idxs=1024,
                            num_idxs_reg=1024,
                            elem_size=D,
                            single_packet=True,
                            queue_num=pc % 4,
                        )
                    dst = acc_lab[:, half * 64:(half + 1) * 64, :]
                    if c == 0:
                        nc.vector.tensor_copy(out=dst, in_=lt[:])
                    else:
                        nc.vector.tensor_add(out=dst, in0=dst, in1=lt[:])

            # rank[i] at [i%128, i//128]; src slots g 0..63, dst slots g 64..127
            rmul = ep.tile([128, 64, D], F32, tag="e16a")
            nc.vector.tensor_mul(out=rmul[:], in0=acc_lab[:, :64, :],
                                 in1=acc_lab[:, 64:, :])
            rank = ep.tile([128, 64], F32, tag="rank")
            nc.vector.reduce_sum(rank[:], rmul[:], axis=mybir.AxisListType.X)

            # ---- beta windows
            bwin_sb = ep.tile([128, 512], I16, tag="bwin")
            nc.sync.dma_start(out=bwin_sb[:], in_=bwin_e[:, :])
            e8_sb = ep.tile([128, 64, D], F32, tag="e16b")
            nc.sync.dma_start(out=e8_sb[:],
                              in_=e8_e[:, :].rearrange("p (g d) -> p g d", d=D))
            bw = ep.tile([128, 64, D], F32, tag="e16c")
            for piece in range(8):
                nc.gpsimd.dma_gather(
                    out_ap=bw[:, piece * 8:(piece + 1) * 8, :],
                    in_ap=beta_e[:, :],
                    idxs_ap=bwin_sb[:, piece * 64:(piece + 1) * 64],
                    num_idxs=1024,
                    num_idxs_reg=1024,
                    elem_size=D,
                    single_packet=True,
                    queue_num=piece % 4,
                )
            bsel = ep.tile([128, 64, D], F32, tag="e16a")
            nc.vector.tensor_mul(out=bsel[:], in0=bw[:], in1=e8_sb[:])
            betad = ep.tile([128, 64], F32, tag="betad")
            nc.vector.reduce_sum(betad[:], bsel[:], axis=mybir.AxisListType.X)

            # ---- broadcast rank to all partitions as a free-dim row
            ident = ep.tile([128, 128], F32, tag="ident")
            make_identity(nc, ident[:])
            tps = psp.tile([64, 128], F32, tag="tp")
            nc.tensor.transpose(out=tps[:], in_=rank[:, :], identity=ident[:])
            tr = ep.tile([64, 128], F32, tag="tr")
            nc.vector.tensor_copy(out=tr[:], in_=tps[:])
            nc.sync.dma_start(
                out=rrow_d[0, :].rearrange("(a b) -> a b", b=128), in_=tr[:])
            rbc = ep.tile([128, NLAB], F32, tag="rbc")
            nc.sync.dma_start(out=rbc[:],
                              in_=rrow_d[0:1, :].to_broadcast([128, NLAB]))

            # ---- outer sums, rows sharded by osel
            osel_sb = ep.tile([128, 8, 64], F32, tag="osel")
            nc.sync.dma_start(out=osel_sb[:],
                              in_=osel_e[:, :].rearrange("p (t g) -> p t g", g=64))
            for t in range(4):
                for pn in range(2):
                    sel = wp.tile([128, 64], F32, tag="osel_t")
                    nc.vector.tensor_mul(out=sel[:], in0=betad[:],
                                         in1=osel_sb[:, pn * 4 + t, :])
                    bval = wp.tile([128, 1], F32, tag="bval")
                    nc.vector.reduce_sum(bval[:], sel[:], axis=mybir.AxisListType.X)
                    for half in range(2):
                        orow = op.tile([128, 2048], F32, tag="orow")
                        nc.vector.tensor_scalar_add(
                            out=orow[:],
                            in0=rbc[:, pn * 4096 + half * 2048:
                                    pn * 4096 + (half + 1) * 2048],
                            scalar1=bval[:, 0:1])
                        nc.sync.dma_start(
                            out=out_e[pn, t * 128:(t + 1) * 128,
                                      half * 2048:(half + 1) * 2048],
                            in_=orow[:])


# ---------------------------------------------------------------- entry
def kernel(emb_weight, beta_weight, alpha, edge_index, edge_label_index,
           num_users, num_items, scaling_factor):
    in_maps, meta = _prep(emb_weight, beta_weight, alpha, edge_index,
                          edge_label_index, num_users, scaling_factor)
    nc = _build(meta)
    nc.finalize()
    res = run_bass_kernel_spmd(nc, in_maps, list(range(NCORE)))
    pos = np.concatenate([res.results[k]["out"][0] for k in range(NCORE)], axis=0)
    neg = np.concatenate([res.results[k]["out"][1] for k in range(NCORE)], axis=0)
    return pos, neg
